# revision 25
# baseline (speedup 1.0000x reference)
"""Causal multi-head attention on 8 Trainium2 NeuronCores.

Problem: B=2, S=4096, D_MODEL=768, H=12, D_HEAD=64, fp32 I/O.

Sharding: (batch, head-group) -> core.  Cores 0-3 take batch 0, cores 4-7
take batch 1; each core computes 3 of the 12 heads for its batch and emits a
partial output [S, D_MODEL] (its heads' contribution to the W_O contraction)
in bf16.  The host sums the 4 partials per batch and adds b_O.

v3 design (vs v2): trace showed PE active 292us / ACT 233us of a 345us span,
with 26us startup, 73us of ACT gaps and a 23us half-clock tail.  Fixes:
  1. Startup: DMA order is idb|wqkv|xT-c0|wv3|...|xT-c1 so the first
     projection can start as soon as chunk 0 lands; the HAM warm-up spins on
     the zc PSUM bank (v2 used fill_ps, so proj #1 queued behind warm-up #40)
     and xT chunks >= 2 are DMA'd just-in-time at the end of window qs-2.
  2. ACT gaps: proj filler thunks split 6 matmuls -> 3x2 so a pop never
     exceeds the per-key-tile ACT slack; vproj split 2x3.
  3. Window-boundary gaps: norm_stage1 is split per head and h0/h1 copies
     are emitted before the h2 key loop, freeing the zab PSUM bank a full
     h2-loop earlier for the next window's AV accumulation.
  4. Junk trim: h1 (and h2's odd key tile) scores are written at shifted
     columns [QCH, 2QCH-vs) so diagonal-window exp calls shrink by vs on
     both ends (~8.5us ACT + ~8.5us PE).
  5. bf16 rank-1 1/r broadcast (f32r rhs streamed at half PE rate).
  6. Tail: anti-throttle dummy matmuls gated on the last window's norm
     chain keep the clock at 2.4GHz through the epilogue.
"""

import numpy as np
import ml_dtypes

B, S, DM, H, DH = 2, 4096, 768, 12, 64
NCORES = 8
GROUPS = 4                  # head-groups per batch
HPC = H // GROUPS           # heads per core = 3
P = 128
QCH = 512                   # psum bank width (fp32)

_BF = ml_dtypes.bfloat16

_cache = {}


def _build(seq_len, use_biases):
    import concourse.bacc as bacc
    import concourse.mybir as mybir
    import concourse.tile as tile

    f32 = mybir.dt.float32
    bf16 = mybir.dt.bfloat16
    Exp = mybir.ActivationFunctionType.Exp
    mult = mybir.AluOpType.mult
    add = mybir.AluOpType.add

    SQ = seq_len
    n_kt = SQ // P               # k tiles
    n_ch = SQ // QCH             # 512-wide chunks
    DSL = DM // P                # contraction slices for the projections
    KPC = QCH // P               # key tiles per chunk (4)

    nc = bacc.Bacc(None, target_bir_lowering=False)

    xT = nc.declare_dram_parameter("xT", [DM, SQ], bf16, isOutput=False)
    # packed projection weights: [Q01 | K01 | Q2K2] = 384 cols
    wqkv = nc.declare_dram_parameter("wqkv", [DM, 384], bf16, isOutput=False)
    # V weights for the direct [keys, z] projection, all 3 heads
    wv3 = nc.declare_dram_parameter("wv3", [DM, HPC * DH], bf16,
                                    isOutput=False)
    wo2 = nc.declare_dram_parameter("wo2", [P, DM], bf16, isOutput=False)
    wos = nc.declare_dram_parameter("wos", [DH, DM], bf16, isOutput=False)
    trimask = nc.declare_dram_parameter("trimask", [P, P], bf16, isOutput=False)
    ident_b = nc.declare_dram_parameter("ident_b", [P, P], bf16, isOutput=False)
    ones_z = nc.declare_dram_parameter("ones_z", [1, DH], bf16, isOutput=False)
    if use_biases:
        bqkv = nc.declare_dram_parameter("bqkv", [P, 3], f32, isOutput=False)
        bvrep = nc.declare_dram_parameter("bvrep", [P, HPC * DH], f32,
                                          isOutput=False)
    out = nc.declare_dram_parameter("out", [SQ, DM], bf16, isOutput=True)

    with tile.TileContext(nc) as tc:
        with (
            tc.tile_pool(name="singles", bufs=1) as singles,
            tc.tile_pool(name="persist", bufs=1) as persist,
            tc.tile_pool(name="nrm_t", bufs=2) as nrm_t,
            tc.tile_pool(name="nrm_k", bufs=6) as nrm_k,
            tc.tile_pool(name="xT_pool", bufs=1) as xT_pool,
            tc.tile_pool(name="s_ps", bufs=2, space="PSUM") as s_ps,
            tc.tile_pool(name="zab_ps", bufs=1, space="PSUM") as zab_ps,
            tc.tile_pool(name="zc_ps", bufs=1, space="PSUM") as zc_ps,
            tc.tile_pool(name="fill_ps", bufs=1, space="PSUM") as fill_ps,
            tc.tile_pool(name="pt_sb", bufs=4) as pt_pool,
            tc.tile_pool(name="o_sb", bufs=8) as o_pool,
        ):
            # ---- constants / weights, ordered for fastest first proj ----
            idb_sb = singles.tile([P, P], bf16)
            nc.sync.dma_start(idb_sb[:], ident_b[:])
            w_sb = singles.tile([P, DSL, 384], bf16, tag="wqkv")
            nc.sync.dma_start(w_sb[:], wqkv.rearrange("(o p) c -> p o c", p=P))
            xT_sb = xT_pool.tile([P, DSL, SQ], bf16)
            nc.sync.dma_start(
                xT_sb[:, :, 0:QCH],
                xT.rearrange("(o p) c -> p o c", p=P)[:, :, 0:QCH])
            wv_sb = singles.tile([P, DSL, HPC * DH], bf16, tag="wv3")
            nc.sync.dma_start(wv_sb[:], wv3.rearrange("(o p) c -> p o c", p=P))
            tri_sb = singles.tile([P, P], bf16)
            nc.sync.dma_start(tri_sb[:], trimask[:])
            ones_sb = singles.tile([1, DH], bf16)
            nc.sync.dma_start(ones_sb[:], ones_z[:])
            nc.sync.dma_start(
                xT_sb[:, :, QCH:2 * QCH],
                xT.rearrange("(o p) c -> p o c", p=P)[:, :, QCH:2 * QCH])
            wo2_sb = singles.tile([P, DM], bf16)
            nc.sync.dma_start(wo2_sb[:], wo2[:])
            wos_sb = singles.tile([DH, DM], bf16)
            nc.sync.dma_start(wos_sb[:], wos[:])
            for c0 in range(2, min(4, n_ch)):
                nc.sync.dma_start(
                    xT_sb[:, :, c0 * QCH:(c0 + 1) * QCH],
                    xT.rearrange("(o p) c -> p o c", p=P)[
                        :, :, c0 * QCH:(c0 + 1) * QCH])
            bias_sb = bv_sb = None
            if use_biases:
                bias_sb = singles.tile([P, 3], f32, tag="bias")
                nc.sync.dma_start(bias_sb[:], bqkv[:])
                bv_sb = singles.tile([P, HPC * DH], f32, tag="bvrep")
                nc.sync.dma_start(bv_sb[:], bvrep[:])

            # ---- persistent activations ----
            QT2 = persist.tile([P, SQ], bf16, tag="QT2")   # heads 0,1 stacked
            KT2 = persist.tile([P, SQ], bf16, tag="KT2")
            # head 2's Q/K live on BOTH partition halves so even/odd key
            # tiles can run concurrently in the two PE row groups
            Q2b = persist.tile([P, SQ], bf16, tag="Q2b")
            K2b = persist.tile([P, SQ], bf16, tag="K2b")
            # V tiles are padded to 128 columns (64 z + ones + 63 zeros) so
            # the AV LDWEIGHTS take the fast-weight-load path (needs 128
            # cols); the junk output rows 65-127 land in unused partitions
            # of the same PSUM banks
            V_sb = persist.tile([P, HPC, n_kt, P], bf16, tag="V")
            Zn2 = persist.tile([P, SQ], bf16, tag="Zn2")   # h0 rows 0-63, h1 64-127
            Zns = persist.tile([DH, SQ], bf16, tag="Zns")  # h2
            # single-instance scratch for the row-sum reciprocal: rows past
            # the written ones are read by the 32x32 block transposes, so
            # they are zeroed once and the tiles reused in place
            r32g = persist.tile([32, QCH], f32, tag="r32g")
            rrTg = persist.tile([32, QCH], f32, tag="rrTg")

            nc.vector.memset(V_sb[:, :, :, DH:DH + 1], 1.0)
            nc.vector.memset(V_sb[:, :, :, DH + 1:], 0.0)
            nc.vector.memset(r32g[:], 1.0)
            nc.vector.memset(rrTg[:], 1.0)

            # HAM warm-up on the zc bank (fill_ps stays free for the first
            # projections); zc's first real use is window 0's h2 AV, long
            # after this drains.  Kept short: the clock grant arrives ~5us
            # after PE activity starts, and dense MACs at full clock burn
            # the HAM utilization budget (16.4us half-clock quanta).
            wup = zc_ps.tile([P, QCH], f32, tag="zc", name="wup")
            for _ in range(40):
                nc.tensor.matmul(wup[:, 0:P], lhsT=idb_sb[:],
                                 rhs=idb_sb[:], start=True, stop=True)

            # ================= projection thunks =================
            # groups: 0=Q01, 1=K01, 2=Q2|K2; V is projected per key-tile
            # directly in [keys, z] layout (lhsT = xT slice).
            def proj_mm(g, c, st, ps_slot):
                """Two of the six contraction matmuls for group g chunk c."""
                cs = slice(c * QCH, (c + 1) * QCH)
                if st == 0:
                    ps_slot[0] = fill_ps.tile([P, QCH], f32, tag="fill",
                                              name="proj_ps")
                ps = ps_slot[0]
                for o in (2 * st, 2 * st + 1):
                    nc.tensor.matmul(
                        ps[:], lhsT=w_sb[:, o, g * P:(g + 1) * P],
                        rhs=xT_sb[:, o, cs],
                        start=(o == 0), stop=(o == DSL - 1))
                if st < 2:
                    return
                if use_biases:
                    def cp(dst, src, brow):
                        nc.vector.tensor_scalar(
                            dst, src, bias_sb[brow, g:g + 1], None, add)
                else:
                    def cp(dst, src, brow=None):
                        nc.vector.tensor_copy(dst, src)
                if g == 0:
                    cp(QT2[:, cs], ps[:], slice(0, P))
                elif g == 1:
                    cp(KT2[:, cs], ps[:], slice(0, P))
                else:
                    cp(Q2b[0:DH, cs], ps[0:DH], slice(0, DH))
                    cp(K2b[DH:P, cs], ps[DH:P], slice(DH, P))
                    nc.sync.dma_start(Q2b[DH:P, cs], Q2b[0:DH, cs])
                    nc.sync.dma_start(K2b[0:DH, cs], K2b[DH:P, cs])

            def vproj_mm(kt, st, ps_slot):
                """Half of V-proj for one key-tile: [128 keys, 192]."""
                if st == 0:
                    ps_slot[0] = fill_ps.tile([P, QCH], f32, tag="fill",
                                              name="vproj_ps")[:, 0:HPC * DH]
                ps = ps_slot[0]
                for o in range(3 * st, 3 * st + 3):
                    nc.tensor.matmul(
                        ps[:], lhsT=xT_sb[:, o, kt * P:(kt + 1) * P],
                        rhs=wv_sb[:, o, :],
                        start=(o == 0), stop=(o == DSL - 1))
                if st == 0:
                    return
                dst = V_sb[:, :, kt, 0:DH]
                src = ps.rearrange("p (h z) -> p h z", z=DH)
                if use_biases:
                    nc.vector.tensor_tensor(
                        dst, src, bv_sb.rearrange("p (h z) -> p h z", z=DH),
                        add)
                else:
                    nc.vector.tensor_copy(dst, src)

            def proj_thunks(c):
                for g in range(3):
                    slot = [None]
                    for st in range(3):
                        yield (lambda g=g, c=c, st=st, slot=slot:
                               proj_mm(g, c, st, slot))
                for j in range(KPC):
                    slot = [None]
                    for st in range(2):
                        yield (lambda kt=c * KPC + j, st=st, slot=slot:
                               vproj_mm(kt, st, slot))

            # ---- normalization helpers (DVE stage + deferred PE stage) ----
            def stage1(zacc):
                """All-DVE per head: copy Z to SBUF (freeing the PSUM bank),
                extract row sums, reciprocal via 32x32 block transposes."""
                zsb = nrm_k.tile([DH, QCH], bf16, tag="zsb", name="zsb")
                nc.vector.tensor_copy(zsb[:], zacc[0:DH, :])
                nc.vector.tensor_copy(r32g[0:1, :], zacc[DH:DH + 1, :])
                rT = nrm_t.tile([32, QCH], f32, tag="rT", name="rT")
                nc.vector.transpose(rT[:], r32g[:])
                nc.vector.reciprocal(
                    rrTg.rearrange("p (j c) -> p j c", c=32)[:, :, 0],
                    rT.rearrange("p (j c) -> p j c", c=32)[:, :, 0])
                rr32 = nrm_t.tile([32, QCH], f32, tag="rr32", name="rr32")
                nc.vector.transpose(rr32[:], rrTg[:])
                rr = nrm_k.tile([1, QCH], bf16, tag="rr", name="rr_sb")
                nc.vector.tensor_copy(rr[:], rr32[0:1, :])
                return (rr, zsb)

            def norm_stage2(h, q0, staged):
                """PE rank-1 broadcast of 1/r, then one DVE multiply."""
                rr_sb, zsb = staged
                rrb = fill_ps.tile([P, QCH], f32, tag="fill",
                                   name="rrb")[0:DH]
                nc.tensor.matmul(rrb[:], lhsT=ones_sb[:],
                                 rhs=rr_sb[:], start=True, stop=True)
                if h == 0:
                    nc.vector.tensor_tensor(
                        Zn2[0:DH, q0:q0 + QCH], zsb[:], rrb[:], mult)
                elif h == 1:
                    t = nrm_k.tile([DH, QCH], bf16, tag="zn1", name="zn1")
                    nc.vector.tensor_tensor(t[:], zsb[:], rrb[:], mult)
                    nc.sync.dma_start(Zn2[DH:P, q0:q0 + QCH], t[:])
                else:
                    nc.vector.tensor_tensor(
                        Zns[:, q0:q0 + QCH], zsb[:], rrb[:], mult)

            # ===== flash: all heads interleaved, one 512-wide window loop ====
            SW = 2 * QCH            # psum score-slot width (tag "S")
            HD = DM // 2
            fills = []

            def oproj_thunks(w, tail=False):
                """O-proj for window w as per-half-tile filler thunks:
                h01 packed (contraction 128) + h2 (contraction 64).  In the
                post-loop tail the score banks are dead, so po rotates
                through s_ps (bufs=2) instead of serializing every
                matmul/cast pair through the single fill bank."""
                thunks = []
                for tt in range(w * (QCH // P), (w + 1) * (QCH // P)):
                    osb = o_pool.tile([P, DM], bf16, tag="osb", name="osb")

                    def th(tt=tt, osb=osb, half=0, tl=tail):
                        if tl:
                            po = s_ps.tile([P, 2 * QCH], f32, tag="S",
                                           name="po_t")[:, 0:HD]
                        else:
                            po = fill_ps.tile([P, QCH], f32, tag="fill",
                                              name="po")[:, 0:HD]
                        hs = slice(half * HD, (half + 1) * HD)
                        nc.tensor.matmul(
                            po[:], lhsT=Zn2[:, tt * P:(tt + 1) * P],
                            rhs=wo2_sb[:, hs], start=True, stop=False)
                        nc.tensor.matmul(
                            po[:], lhsT=Zns[:, tt * P:(tt + 1) * P],
                            rhs=wos_sb[:, hs], start=False, stop=True)
                        nc.vector.tensor_copy(osb[:, hs], po[:])
                        if half == 1:
                            nc.sync.dma_start(out[tt * P:(tt + 1) * P, :],
                                              osb[:])

                    thunks.append(th)
                    thunks.append(
                        lambda tt=tt, osb=osb, th=th: th(tt, osb, 1))
                return thunks

            # Ascending windows; proj chunks {0,1} up front, chunk qs+2
            # rides window qs's fills.  stage2/oproj for window qs-1 also
            # ride window qs, with oproj queued after proj so its stage2 /
            # Zn-shift-DMA inputs are settled by the time it pops.
            # chunks 0-3 are projected up front: ACT is idle during the
            # (half-clock) startup anyway, and this keeps the early windows'
            # filler load from starving the score matmuls that feed exp
            for c in range(min(4, n_ch)):
                for th in proj_thunks(c):
                    th()

            staged = {}
            prev = None
            for qs in range(n_ch):
                q0 = qs * QCH
                # just-in-time xT stream: chunk qs+3's DMA is emitted one
                # window before its projection fills are queued below
                if 4 <= qs + 3 < n_ch:
                    c = qs + 3
                    nc.sync.dma_start(
                        xT_sb[:, :, c * QCH:(c + 1) * QCH],
                        xT.rearrange("(o p) c -> p o c", p=P)[
                            :, :, c * QCH:(c + 1) * QCH])
                if prev is not None:
                    for h in range(HPC):
                        fills.append(
                            (lambda h=h, q=prev * QCH, st=staged[(prev, h)]:
                             norm_stage2(h, q, st)))
                # deadline-scheduled projection fills: chunk qs+2 is needed
                # by window qs+2, so queueing here lands the load on the
                # big late windows that have pop slack, not windows 1-3
                if 4 <= qs + 2 < n_ch:
                    fills.extend(proj_thunks(qs + 2))
                if prev is not None:
                    fills.extend(oproj_thunks(prev))

                zab = zab_ps.tile([P, 2 * QCH], f32, tag="zab",
                                  name="zab")
                za = zab[:, 0:QCH]
                zb = zab[:, QCH:2 * QCH]
                zc = zc_ps.tile([P, QCH], f32, tag="zc", name="zc")
                nk = KPC * qs + KPC
                # --- heads 0,1: concurrent scores in two PE row groups;
                # h1 columns are shifted by -vs on diagonal tiles so the
                # joint exp call is junk-free on both ends ---
                for ki in range(nk):
                    vs = max(0, P * ki - q0)
                    ssc = s_ps.tile([P, SW], f32, tag="S", name="ssc")
                    nc.tensor.matmul(
                        ssc[:, vs:QCH],
                        lhsT=KT2[0:DH, ki * P:(ki + 1) * P],
                        rhs=QT2[0:DH, q0 + vs:q0 + QCH],
                        start=True, stop=True)
                    nc.tensor.matmul(
                        ssc[:, QCH:2 * QCH - vs],
                        lhsT=KT2[DH:P, ki * P:(ki + 1) * P],
                        rhs=QT2[DH:P, q0 + vs:q0 + QCH],
                        start=True, stop=True)
                    pt = pt_pool.tile([P, 2 * QCH], bf16, tag="PT",
                                      name="pt")
                    nc.scalar.activation(
                        pt[:, vs:2 * QCH - vs], ssc[:, vs:2 * QCH - vs],
                        Exp, scale=0.125)
                    if ki >= KPC * qs:  # diagonal tile: mask both heads
                        nc.vector.tensor_tensor(
                            pt[:, vs:vs + P], pt[:, vs:vs + P],
                            tri_sb[:], mult)
                        nc.vector.tensor_tensor(
                            pt[:, QCH:QCH + P], pt[:, QCH:QCH + P],
                            tri_sb[:], mult)
                    nc.tensor.matmul(
                        za[:, vs:QCH], lhsT=V_sb[:, 0, ki, :],
                        rhs=pt[:, vs:QCH],
                        start=(ki == 0), stop=(ki == nk - 1))
                    nc.tensor.matmul(
                        zb[:, vs:QCH], lhsT=V_sb[:, 1, ki, :],
                        rhs=pt[:, QCH:2 * QCH - vs],
                        start=(ki == 0), stop=(ki == nk - 1))
                    if fills:
                        fills.pop(0)()
                # za/zb are complete: free the zab bank for the next window
                # before the h2 loop runs (DVE copies overlap h2's ACT time)
                staged[(qs, 0)] = stage1(za)
                staged[(qs, 1)] = stage1(zb)
                if qs == n_ch - 1:
                    # last window: h0/h1 stage2 ride the h2 loop's fill pops
                    # so the post-loop tail only carries the h2 chain
                    for h in range(2):
                        fills.append(
                            (lambda h=h, q=q0, st=staged[(qs, h)]:
                             norm_stage2(h, q, st)))
                # --- head 2: two key-tiles per exp call ---
                for kj in range(0, nk, 2):
                    vs0 = max(0, P * kj - q0)
                    vs1 = max(0, P * (kj + 1) - q0)
                    ssc = s_ps.tile([P, SW], f32, tag="S", name="ssc2")
                    nc.tensor.matmul(
                        ssc[:, vs0:QCH],
                        lhsT=K2b[0:DH, kj * P:(kj + 1) * P],
                        rhs=Q2b[0:DH, q0 + vs0:q0 + QCH],
                        start=True, stop=True)
                    nc.tensor.matmul(
                        ssc[:, QCH:2 * QCH - vs1],
                        lhsT=K2b[DH:P, (kj + 1) * P:(kj + 2) * P],
                        rhs=Q2b[DH:P, q0 + vs1:q0 + QCH],
                        start=True, stop=True)
                    pt = pt_pool.tile([P, 2 * QCH], bf16, tag="PT2",
                                      name="pt2")
                    nc.scalar.activation(
                        pt[:, vs0:2 * QCH - vs1], ssc[:, vs0:2 * QCH - vs1],
                        Exp, scale=0.125)
                    if kj >= KPC * qs:  # both tiles on the diagonal
                        nc.vector.tensor_tensor(
                            pt[:, vs0:vs0 + P], pt[:, vs0:vs0 + P],
                            tri_sb[:], mult)
                        nc.vector.tensor_tensor(
                            pt[:, QCH:QCH + P], pt[:, QCH:QCH + P],
                            tri_sb[:], mult)
                    nc.tensor.matmul(
                        zc[:, vs0:QCH], lhsT=V_sb[:, 2, kj, :],
                        rhs=pt[:, vs0:QCH],
                        start=(kj == 0), stop=False)
                    nc.tensor.matmul(
                        zc[:, vs1:QCH], lhsT=V_sb[:, 2, kj + 1, :],
                        rhs=pt[:, QCH:2 * QCH - vs1],
                        start=False, stop=(kj + 1 == nk - 1))
                    if fills:
                        fills.pop(0)()
                staged[(qs, 2)] = stage1(zc)
                prev = qs
            while fills:
                fills.pop(0)()
            norm_stage2(2, prev * QCH, staged[(prev, 2)])
            for th in oproj_thunks(prev, tail=True):
                th()

    nc.compile()
    return nc


def _prep_inputs(inputs, seq_len, use_biases):
    x = np.asarray(inputs["normalized_resid_pre"], dtype=np.float32)
    WQ = np.asarray(inputs["W_Q"], dtype=np.float32)
    WK = np.asarray(inputs["W_K"], dtype=np.float32)
    WV = np.asarray(inputs["W_V"], dtype=np.float32)
    WO = np.asarray(inputs["W_O"], dtype=np.float32)

    tri = np.triu(np.ones((P, P), np.float32)).astype(_BF)  # keep j >= p
    idb = np.eye(P, dtype=np.float32).astype(_BF)
    onz = np.ones((1, DH), np.float32).astype(_BF)

    in_maps = []
    for c in range(NCORES):
        b, g = divmod(c, GROUPS)
        hs = slice(g * HPC, (g + 1) * HPC)
        wq = WQ[hs]   # [3, DM, DH]
        wk = WK[hs]
        wv = WV[hs]
        wo = WO[hs]   # [3, DH, DM]
        # packed groups: [Q01 | K01 | Q2K2] -> [DM, 384]
        wqkv = np.concatenate([
            wq[0], wq[1], wk[0], wk[1], wq[2], wk[2],
        ], axis=1)
        wv3 = np.concatenate([wv[0], wv[1], wv[2]], axis=1)
        m = {
            "xT": np.ascontiguousarray(x[b, :seq_len].T).astype(_BF),
            "wqkv": np.ascontiguousarray(wqkv).astype(_BF),
            "wv3": np.ascontiguousarray(wv3).astype(_BF),
            "wo2": np.ascontiguousarray(
                np.concatenate([wo[0], wo[1]], axis=0)).astype(_BF),
            "wos": np.ascontiguousarray(wo[2]).astype(_BF),
            "trimask": tri,
            "ident_b": idb,
            "ones_z": onz,
        }
        if use_biases:
            bq = np.asarray(inputs["b_Q"], np.float32)[hs]
            bk = np.asarray(inputs["b_K"], np.float32)[hs]
            bv = np.asarray(inputs["b_V"], np.float32)[hs]
            bias = np.zeros((P, 3), np.float32)
            bias[:, 0] = np.concatenate([bq[0], bq[1]])
            bias[:, 1] = np.concatenate([bk[0], bk[1]])
            bias[:, 2] = np.concatenate([bq[2], bk[2]])
            m["bqkv"] = bias
            m["bvrep"] = np.broadcast_to(
                bv.reshape(1, HPC * DH), (P, HPC * DH)).copy()
        in_maps.append(m)
    return in_maps


TRACE = False          # test.py can flip this to get exec_time_ns
last_result = None     # BassKernelResults of the most recent run


def kernel(seq_len=S, **inputs):
    global last_result
    from concourse.bass_utils import run_bass_kernel_spmd

    use_biases = any(
        np.any(np.asarray(inputs[k]) != 0) for k in ("b_Q", "b_K", "b_V"))

    key = (seq_len, use_biases)
    if key not in _cache:
        _cache[key] = _build(seq_len, use_biases)
    nc = _cache[key]

    in_maps = _prep_inputs(inputs, seq_len, use_biases)
    res = run_bass_kernel_spmd(nc, in_maps, core_ids=list(range(NCORES)),
                               trace=TRACE)
    last_result = res

    b_O = np.asarray(inputs["b_O"], dtype=np.float32)
    out = np.zeros((B, seq_len, DM), np.float32)
    for c in range(NCORES):
        b = c // GROUPS
        out[b] += np.asarray(res.results[c]["out"], dtype=np.float32)
    out += b_O[None, None, :]
    return out


# revision 26
# speedup vs baseline: 1.0246x; 1.0246x over previous
"""Causal multi-head attention on 8 Trainium2 NeuronCores.

Problem: B=2, S=4096, D_MODEL=768, H=12, D_HEAD=64, fp32 I/O.

Sharding: (batch, head-group) -> core.  Cores 0-3 take batch 0, cores 4-7
take batch 1; each core computes 3 of the 12 heads for its batch and emits a
partial output [S, D_MODEL] (its heads' contribution to the W_O contraction)
in bf16.  The host sums the 4 partials per batch and adds b_O.

v3 design (vs v2): trace showed PE active 292us / ACT 233us of a 345us span,
with 26us startup, 73us of ACT gaps and a 23us half-clock tail.  Fixes:
  1. Startup: DMA order is idb|wqkv|xT-c0|wv3|...|xT-c1 so the first
     projection can start as soon as chunk 0 lands; the HAM warm-up spins on
     the zc PSUM bank (v2 used fill_ps, so proj #1 queued behind warm-up #40)
     and xT chunks >= 2 are DMA'd just-in-time at the end of window qs-2.
  2. ACT gaps: proj filler thunks split 6 matmuls -> 3x2 so a pop never
     exceeds the per-key-tile ACT slack; vproj split 2x3.
  3. Window-boundary gaps: norm_stage1 is split per head and h0/h1 copies
     are emitted before the h2 key loop, freeing the zab PSUM bank a full
     h2-loop earlier for the next window's AV accumulation.
  4. Junk trim: h1 (and h2's odd key tile) scores are written at shifted
     columns [QCH, 2QCH-vs) so diagonal-window exp calls shrink by vs on
     both ends (~8.5us ACT + ~8.5us PE).
  5. bf16 rank-1 1/r broadcast (f32r rhs streamed at half PE rate).
  6. Tail: anti-throttle dummy matmuls gated on the last window's norm
     chain keep the clock at 2.4GHz through the epilogue.
"""

import numpy as np
import ml_dtypes

B, S, DM, H, DH = 2, 4096, 768, 12, 64
NCORES = 8
GROUPS = 4                  # head-groups per batch
HPC = H // GROUPS           # heads per core = 3
P = 128
QCH = 512                   # psum bank width (fp32)

_BF = ml_dtypes.bfloat16

_cache = {}


def _build(seq_len, use_biases):
    import concourse.bacc as bacc
    import concourse.mybir as mybir
    import concourse.tile as tile

    f32 = mybir.dt.float32
    bf16 = mybir.dt.bfloat16
    Exp = mybir.ActivationFunctionType.Exp
    mult = mybir.AluOpType.mult
    add = mybir.AluOpType.add

    SQ = seq_len
    n_kt = SQ // P               # k tiles
    n_ch = SQ // QCH             # 512-wide chunks
    DSL = DM // P                # contraction slices for the projections
    KPC = QCH // P               # key tiles per chunk (4)

    nc = bacc.Bacc(None, target_bir_lowering=False)

    xT = nc.declare_dram_parameter("xT", [DM, SQ], bf16, isOutput=False)
    # packed projection weights: [Q01 | K01 | Q2K2] = 384 cols
    wqkv = nc.declare_dram_parameter("wqkv", [DM, 384], bf16, isOutput=False)
    # V weights for the direct [keys, z] projection, all 3 heads
    wv3 = nc.declare_dram_parameter("wv3", [DM, HPC * DH], bf16,
                                    isOutput=False)
    wo2 = nc.declare_dram_parameter("wo2", [P, DM], bf16, isOutput=False)
    wos = nc.declare_dram_parameter("wos", [DH, DM], bf16, isOutput=False)
    trimask = nc.declare_dram_parameter("trimask", [P, P], bf16, isOutput=False)
    ident_b = nc.declare_dram_parameter("ident_b", [P, P], bf16, isOutput=False)
    ones_z = nc.declare_dram_parameter("ones_z", [1, DH], bf16, isOutput=False)
    if use_biases:
        bqkv = nc.declare_dram_parameter("bqkv", [P, 3], f32, isOutput=False)
        bvrep = nc.declare_dram_parameter("bvrep", [P, HPC * DH], f32,
                                          isOutput=False)
    out = nc.declare_dram_parameter("out", [SQ, DM], bf16, isOutput=True)

    with tile.TileContext(nc) as tc:
        with (
            tc.tile_pool(name="singles", bufs=1) as singles,
            tc.tile_pool(name="persist", bufs=1) as persist,
            tc.tile_pool(name="nrm_t", bufs=2) as nrm_t,
            tc.tile_pool(name="nrm_k", bufs=6) as nrm_k,
            tc.tile_pool(name="xT_pool", bufs=1) as xT_pool,
            tc.tile_pool(name="s_ps", bufs=2, space="PSUM") as s_ps,
            tc.tile_pool(name="zab_ps", bufs=1, space="PSUM") as zab_ps,
            tc.tile_pool(name="zc_ps", bufs=1, space="PSUM") as zc_ps,
            tc.tile_pool(name="fill_ps", bufs=1, space="PSUM") as fill_ps,
            tc.tile_pool(name="pt_sb", bufs=4) as pt_pool,
            tc.tile_pool(name="o_sb", bufs=8) as o_pool,
        ):
            # ---- constants / weights, ordered for fastest first proj ----
            idb_sb = singles.tile([P, P], bf16)
            nc.sync.dma_start(idb_sb[:], ident_b[:])
            w_sb = singles.tile([P, DSL, 384], bf16, tag="wqkv")
            nc.sync.dma_start(w_sb[:], wqkv.rearrange("(o p) c -> p o c", p=P))
            xT_sb = xT_pool.tile([P, DSL, SQ], bf16)
            nc.sync.dma_start(
                xT_sb[:, :, 0:QCH],
                xT.rearrange("(o p) c -> p o c", p=P)[:, :, 0:QCH])
            wv_sb = singles.tile([P, DSL, HPC * DH], bf16, tag="wv3")
            nc.sync.dma_start(wv_sb[:], wv3.rearrange("(o p) c -> p o c", p=P))
            tri_sb = singles.tile([P, P], bf16)
            nc.sync.dma_start(tri_sb[:], trimask[:])
            ones_sb = singles.tile([1, DH], bf16)
            nc.sync.dma_start(ones_sb[:], ones_z[:])
            nc.sync.dma_start(
                xT_sb[:, :, QCH:2 * QCH],
                xT.rearrange("(o p) c -> p o c", p=P)[:, :, QCH:2 * QCH])
            wo2_sb = singles.tile([P, DM], bf16)
            nc.sync.dma_start(wo2_sb[:], wo2[:])
            wos_sb = singles.tile([DH, DM], bf16)
            nc.sync.dma_start(wos_sb[:], wos[:])
            for c0 in range(2, min(4, n_ch)):
                nc.sync.dma_start(
                    xT_sb[:, :, c0 * QCH:(c0 + 1) * QCH],
                    xT.rearrange("(o p) c -> p o c", p=P)[
                        :, :, c0 * QCH:(c0 + 1) * QCH])
            bias_sb = bv_sb = None
            if use_biases:
                bias_sb = singles.tile([P, 3], f32, tag="bias")
                nc.sync.dma_start(bias_sb[:], bqkv[:])
                bv_sb = singles.tile([P, HPC * DH], f32, tag="bvrep")
                nc.sync.dma_start(bv_sb[:], bvrep[:])

            # ---- persistent activations ----
            QT2 = persist.tile([P, SQ], bf16, tag="QT2")   # heads 0,1 stacked
            KT2 = persist.tile([P, SQ], bf16, tag="KT2")
            # head 2's Q/K live on BOTH partition halves so even/odd key
            # tiles can run concurrently in the two PE row groups
            Q2b = persist.tile([P, SQ], bf16, tag="Q2b")
            K2b = persist.tile([P, SQ], bf16, tag="K2b")
            # V tiles are padded to 128 columns (64 z + ones + 63 zeros) so
            # the AV LDWEIGHTS take the fast-weight-load path (needs 128
            # cols); the junk output rows 65-127 land in unused partitions
            # of the same PSUM banks
            V_sb = persist.tile([P, HPC, n_kt, P], bf16, tag="V")
            Zn2 = persist.tile([P, SQ], bf16, tag="Zn2")   # h0 rows 0-63, h1 64-127
            Zns = persist.tile([DH, SQ], bf16, tag="Zns")  # h2
            # single-instance scratch for the row-sum reciprocal: rows past
            # the written ones are read by the 32x32 block transposes, so
            # they are zeroed once and the tiles reused in place
            r32g = persist.tile([32, QCH], f32, tag="r32g")
            rrTg = persist.tile([32, QCH], f32, tag="rrTg")

            nc.vector.memset(V_sb[:, :, :, DH:DH + 1], 1.0)
            nc.vector.memset(V_sb[:, :, :, DH + 1:], 0.0)
            nc.vector.memset(r32g[:], 1.0)
            nc.vector.memset(rrTg[:], 1.0)

            # HAM warm-up on the zc bank (fill_ps stays free for the first
            # projections); zc's first real use is window 0's h2 AV, long
            # after this drains.  Kept short: the clock grant arrives ~5us
            # after PE activity starts, and dense MACs at full clock burn
            # the HAM utilization budget (16.4us half-clock quanta).
            wup = zc_ps.tile([P, QCH], f32, tag="zc", name="wup")
            for _ in range(40):
                nc.tensor.matmul(wup[:, 0:P], lhsT=idb_sb[:],
                                 rhs=idb_sb[:], start=True, stop=True)

            # ================= projection thunks =================
            # groups: 0=Q01, 1=K01, 2=Q2|K2; V is projected per key-tile
            # directly in [keys, z] layout (lhsT = xT slice).
            def proj_mm(g, c, st, ps_slot):
                """Two of the six contraction matmuls for group g chunk c."""
                cs = slice(c * QCH, (c + 1) * QCH)
                if st == 0:
                    ps_slot[0] = fill_ps.tile([P, QCH], f32, tag="fill",
                                              name="proj_ps")
                ps = ps_slot[0]
                for o in (2 * st, 2 * st + 1):
                    nc.tensor.matmul(
                        ps[:], lhsT=w_sb[:, o, g * P:(g + 1) * P],
                        rhs=xT_sb[:, o, cs],
                        start=(o == 0), stop=(o == DSL - 1))
                if st < 2:
                    return
                if use_biases:
                    def cp(dst, src, brow):
                        nc.vector.tensor_scalar(
                            dst, src, bias_sb[brow, g:g + 1], None, add)
                else:
                    def cp(dst, src, brow=None):
                        nc.vector.tensor_copy(dst, src)
                if g == 0:
                    cp(QT2[:, cs], ps[:], slice(0, P))
                elif g == 1:
                    cp(KT2[:, cs], ps[:], slice(0, P))
                else:
                    cp(Q2b[0:DH, cs], ps[0:DH], slice(0, DH))
                    cp(K2b[DH:P, cs], ps[DH:P], slice(DH, P))
                    nc.sync.dma_start(Q2b[DH:P, cs], Q2b[0:DH, cs])
                    nc.sync.dma_start(K2b[0:DH, cs], K2b[DH:P, cs])

            def vproj_mm(kt, st, ps_slot):
                """Half of V-proj for one key-tile: [128 keys, 192]."""
                if st == 0:
                    ps_slot[0] = fill_ps.tile([P, QCH], f32, tag="fill",
                                              name="vproj_ps")[:, 0:HPC * DH]
                ps = ps_slot[0]
                for o in range(3 * st, 3 * st + 3):
                    nc.tensor.matmul(
                        ps[:], lhsT=xT_sb[:, o, kt * P:(kt + 1) * P],
                        rhs=wv_sb[:, o, :],
                        start=(o == 0), stop=(o == DSL - 1))
                if st == 0:
                    return
                dst = V_sb[:, :, kt, 0:DH]
                src = ps.rearrange("p (h z) -> p h z", z=DH)
                if use_biases:
                    nc.vector.tensor_tensor(
                        dst, src, bv_sb.rearrange("p (h z) -> p h z", z=DH),
                        add)
                else:
                    nc.vector.tensor_copy(dst, src)

            def proj_thunks(c):
                for g in range(3):
                    slot = [None]
                    for st in range(3):
                        yield (lambda g=g, c=c, st=st, slot=slot:
                               proj_mm(g, c, st, slot))
                for j in range(KPC):
                    slot = [None]
                    for st in range(2):
                        yield (lambda kt=c * KPC + j, st=st, slot=slot:
                               vproj_mm(kt, st, slot))

            # ---- normalization helpers (DVE stage + deferred PE stage) ----
            def stage1(zacc):
                """All-DVE per head: copy Z to SBUF (freeing the PSUM bank),
                extract row sums, reciprocal via 32x32 block transposes."""
                zsb = nrm_k.tile([DH, QCH], bf16, tag="zsb", name="zsb")
                nc.vector.tensor_copy(zsb[:], zacc[0:DH, :])
                nc.vector.tensor_copy(r32g[0:1, :], zacc[DH:DH + 1, :])
                rT = nrm_t.tile([32, QCH], f32, tag="rT", name="rT")
                nc.vector.transpose(rT[:], r32g[:])
                nc.vector.reciprocal(
                    rrTg.rearrange("p (j c) -> p j c", c=32)[:, :, 0],
                    rT.rearrange("p (j c) -> p j c", c=32)[:, :, 0])
                rr32 = nrm_t.tile([32, QCH], f32, tag="rr32", name="rr32")
                nc.vector.transpose(rr32[:], rrTg[:])
                rr = nrm_k.tile([1, QCH], bf16, tag="rr", name="rr_sb")
                nc.vector.tensor_copy(rr[:], rr32[0:1, :])
                return (rr, zsb)

            def norm_stage2(h, q0, staged):
                """PE rank-1 broadcast of 1/r, then one DVE multiply."""
                rr_sb, zsb = staged
                rrb = fill_ps.tile([P, QCH], f32, tag="fill",
                                   name="rrb")[0:DH]
                nc.tensor.matmul(rrb[:], lhsT=ones_sb[:],
                                 rhs=rr_sb[:], start=True, stop=True)
                if h == 0:
                    nc.vector.tensor_tensor(
                        Zn2[0:DH, q0:q0 + QCH], zsb[:], rrb[:], mult)
                elif h == 1:
                    t = nrm_k.tile([DH, QCH], bf16, tag="zn1", name="zn1")
                    nc.vector.tensor_tensor(t[:], zsb[:], rrb[:], mult)
                    nc.sync.dma_start(Zn2[DH:P, q0:q0 + QCH], t[:])
                else:
                    nc.vector.tensor_tensor(
                        Zns[:, q0:q0 + QCH], zsb[:], rrb[:], mult)

            # ===== flash: all heads interleaved, one 512-wide window loop ====
            SW = 2 * QCH            # psum score-slot width (tag "S")
            HD = DM // 2
            fills = []

            def oproj_thunks(w, tail=False):
                """O-proj for window w as per-half-tile filler thunks:
                h01 packed (contraction 128) + h2 (contraction 64).  In the
                post-loop tail the score banks are dead, so po rotates
                through s_ps (bufs=2) instead of serializing every
                matmul/cast pair through the single fill bank."""
                thunks = []
                for tt in range(w * (QCH // P), (w + 1) * (QCH // P)):
                    osb = o_pool.tile([P, DM], bf16, tag="osb", name="osb")

                    def th(tt=tt, osb=osb, half=0, tl=tail):
                        if tl:
                            po = s_ps.tile([P, 2 * QCH], f32, tag="S",
                                           name="po_t")[:, 0:HD]
                        else:
                            po = fill_ps.tile([P, QCH], f32, tag="fill",
                                              name="po")[:, 0:HD]
                        hs = slice(half * HD, (half + 1) * HD)
                        nc.tensor.matmul(
                            po[:], lhsT=Zn2[:, tt * P:(tt + 1) * P],
                            rhs=wo2_sb[:, hs], start=True, stop=False)
                        nc.tensor.matmul(
                            po[:], lhsT=Zns[:, tt * P:(tt + 1) * P],
                            rhs=wos_sb[:, hs], start=False, stop=True)
                        nc.vector.tensor_copy(osb[:, hs], po[:])
                        if half == 1:
                            nc.sync.dma_start(out[tt * P:(tt + 1) * P, :],
                                              osb[:])

                    thunks.append(th)
                    thunks.append(
                        lambda tt=tt, osb=osb, th=th: th(tt, osb, 1))
                return thunks

            # Ascending windows; proj chunks {0,1} up front, chunk qs+2
            # rides window qs's fills.  stage2/oproj for window qs-1 also
            # ride window qs, with oproj queued after proj so its stage2 /
            # Zn-shift-DMA inputs are settled by the time it pops.
            # chunks 0-3 are projected up front: ACT is idle during the
            # (half-clock) startup anyway, and this keeps the early windows'
            # filler load from starving the score matmuls that feed exp
            for c in range(min(4, n_ch)):
                for th in proj_thunks(c):
                    th()

            staged = {}
            prev = None
            for qs in range(n_ch):
                q0 = qs * QCH
                # just-in-time xT stream: chunk qs+4's DMA is emitted here,
                # before its projection fills are queued below
                if qs + 4 < n_ch:
                    c = qs + 4
                    nc.sync.dma_start(
                        xT_sb[:, :, c * QCH:(c + 1) * QCH],
                        xT.rearrange("(o p) c -> p o c", p=P)[
                            :, :, c * QCH:(c + 1) * QCH])
                if prev is not None:
                    for h in range(HPC):
                        fills.append(
                            (lambda h=h, q=prev * QCH, st=staged[(prev, h)]:
                             norm_stage2(h, q, st)))
                if qs + 4 < n_ch:
                    fills.extend(proj_thunks(qs + 4))
                if prev is not None:
                    fills.extend(oproj_thunks(prev))

                zab = zab_ps.tile([P, 2 * QCH], f32, tag="zab",
                                  name="zab")
                za = zab[:, 0:QCH]
                zb = zab[:, QCH:2 * QCH]
                zc = zc_ps.tile([P, QCH], f32, tag="zc", name="zc")
                nk = KPC * qs + KPC
                # --- heads 0,1: concurrent scores in two PE row groups;
                # h1 columns are shifted by -vs on diagonal tiles so the
                # joint exp call is junk-free on both ends ---
                for ki in range(nk):
                    vs = max(0, P * ki - q0)
                    ssc = s_ps.tile([P, SW], f32, tag="S", name="ssc")
                    nc.tensor.matmul(
                        ssc[:, vs:QCH],
                        lhsT=KT2[0:DH, ki * P:(ki + 1) * P],
                        rhs=QT2[0:DH, q0 + vs:q0 + QCH],
                        start=True, stop=True)
                    nc.tensor.matmul(
                        ssc[:, QCH:2 * QCH - vs],
                        lhsT=KT2[DH:P, ki * P:(ki + 1) * P],
                        rhs=QT2[DH:P, q0 + vs:q0 + QCH],
                        start=True, stop=True)
                    pt = pt_pool.tile([P, 2 * QCH], bf16, tag="PT",
                                      name="pt")
                    nc.scalar.activation(
                        pt[:, vs:2 * QCH - vs], ssc[:, vs:2 * QCH - vs],
                        Exp, scale=0.125)
                    if ki >= KPC * qs:  # diagonal tile: mask both heads
                        nc.vector.tensor_tensor(
                            pt[:, vs:vs + P], pt[:, vs:vs + P],
                            tri_sb[:], mult)
                        nc.vector.tensor_tensor(
                            pt[:, QCH:QCH + P], pt[:, QCH:QCH + P],
                            tri_sb[:], mult)
                    nc.tensor.matmul(
                        za[:, vs:QCH], lhsT=V_sb[:, 0, ki, :],
                        rhs=pt[:, vs:QCH],
                        start=(ki == 0), stop=(ki == nk - 1))
                    nc.tensor.matmul(
                        zb[:, vs:QCH], lhsT=V_sb[:, 1, ki, :],
                        rhs=pt[:, QCH:2 * QCH - vs],
                        start=(ki == 0), stop=(ki == nk - 1))
                    if fills:
                        fills.pop(0)()
                # za/zb are complete: free the zab bank for the next window
                # before the h2 loop runs (DVE copies overlap h2's ACT time)
                staged[(qs, 0)] = stage1(za)
                staged[(qs, 1)] = stage1(zb)
                if qs == n_ch - 1:
                    # last window: h0/h1 stage2 ride the h2 loop's fill pops
                    # so the post-loop tail only carries the h2 chain
                    for h in range(2):
                        fills.append(
                            (lambda h=h, q=q0, st=staged[(qs, h)]:
                             norm_stage2(h, q, st)))
                # --- head 2: two key-tiles per exp call ---
                for kj in range(0, nk, 2):
                    vs0 = max(0, P * kj - q0)
                    vs1 = max(0, P * (kj + 1) - q0)
                    ssc = s_ps.tile([P, SW], f32, tag="S", name="ssc2")
                    nc.tensor.matmul(
                        ssc[:, vs0:QCH],
                        lhsT=K2b[0:DH, kj * P:(kj + 1) * P],
                        rhs=Q2b[0:DH, q0 + vs0:q0 + QCH],
                        start=True, stop=True)
                    nc.tensor.matmul(
                        ssc[:, QCH:2 * QCH - vs1],
                        lhsT=K2b[DH:P, (kj + 1) * P:(kj + 2) * P],
                        rhs=Q2b[DH:P, q0 + vs1:q0 + QCH],
                        start=True, stop=True)
                    pt = pt_pool.tile([P, 2 * QCH], bf16, tag="PT2",
                                      name="pt2")
                    nc.scalar.activation(
                        pt[:, vs0:2 * QCH - vs1], ssc[:, vs0:2 * QCH - vs1],
                        Exp, scale=0.125)
                    if kj >= KPC * qs:  # both tiles on the diagonal
                        nc.vector.tensor_tensor(
                            pt[:, vs0:vs0 + P], pt[:, vs0:vs0 + P],
                            tri_sb[:], mult)
                        nc.vector.tensor_tensor(
                            pt[:, QCH:QCH + P], pt[:, QCH:QCH + P],
                            tri_sb[:], mult)
                    nc.tensor.matmul(
                        zc[:, vs0:QCH], lhsT=V_sb[:, 2, kj, :],
                        rhs=pt[:, vs0:QCH],
                        start=(kj == 0), stop=False)
                    nc.tensor.matmul(
                        zc[:, vs1:QCH], lhsT=V_sb[:, 2, kj + 1, :],
                        rhs=pt[:, QCH:2 * QCH - vs1],
                        start=False, stop=(kj + 1 == nk - 1))
                    if fills:
                        fills.pop(0)()
                staged[(qs, 2)] = stage1(zc)
                prev = qs
            while fills:
                fills.pop(0)()
            norm_stage2(2, prev * QCH, staged[(prev, 2)])
            for th in oproj_thunks(prev, tail=True):
                th()

    nc.compile()
    return nc


def _prep_inputs(inputs, seq_len, use_biases):
    x = np.asarray(inputs["normalized_resid_pre"], dtype=np.float32)
    WQ = np.asarray(inputs["W_Q"], dtype=np.float32)
    WK = np.asarray(inputs["W_K"], dtype=np.float32)
    WV = np.asarray(inputs["W_V"], dtype=np.float32)
    WO = np.asarray(inputs["W_O"], dtype=np.float32)

    tri = np.triu(np.ones((P, P), np.float32)).astype(_BF)  # keep j >= p
    idb = np.eye(P, dtype=np.float32).astype(_BF)
    onz = np.ones((1, DH), np.float32).astype(_BF)

    in_maps = []
    for c in range(NCORES):
        b, g = divmod(c, GROUPS)
        hs = slice(g * HPC, (g + 1) * HPC)
        wq = WQ[hs]   # [3, DM, DH]
        wk = WK[hs]
        wv = WV[hs]
        wo = WO[hs]   # [3, DH, DM]
        # packed groups: [Q01 | K01 | Q2K2] -> [DM, 384]
        wqkv = np.concatenate([
            wq[0], wq[1], wk[0], wk[1], wq[2], wk[2],
        ], axis=1)
        wv3 = np.concatenate([wv[0], wv[1], wv[2]], axis=1)
        m = {
            "xT": np.ascontiguousarray(x[b, :seq_len].T).astype(_BF),
            "wqkv": np.ascontiguousarray(wqkv).astype(_BF),
            "wv3": np.ascontiguousarray(wv3).astype(_BF),
            "wo2": np.ascontiguousarray(
                np.concatenate([wo[0], wo[1]], axis=0)).astype(_BF),
            "wos": np.ascontiguousarray(wo[2]).astype(_BF),
            "trimask": tri,
            "ident_b": idb,
            "ones_z": onz,
        }
        if use_biases:
            bq = np.asarray(inputs["b_Q"], np.float32)[hs]
            bk = np.asarray(inputs["b_K"], np.float32)[hs]
            bv = np.asarray(inputs["b_V"], np.float32)[hs]
            bias = np.zeros((P, 3), np.float32)
            bias[:, 0] = np.concatenate([bq[0], bq[1]])
            bias[:, 1] = np.concatenate([bk[0], bk[1]])
            bias[:, 2] = np.concatenate([bq[2], bk[2]])
            m["bqkv"] = bias
            m["bvrep"] = np.broadcast_to(
                bv.reshape(1, HPC * DH), (P, HPC * DH)).copy()
        in_maps.append(m)
    return in_maps


TRACE = False          # test.py can flip this to get exec_time_ns
last_result = None     # BassKernelResults of the most recent run


def kernel(seq_len=S, **inputs):
    global last_result
    from concourse.bass_utils import run_bass_kernel_spmd

    use_biases = any(
        np.any(np.asarray(inputs[k]) != 0) for k in ("b_Q", "b_K", "b_V"))

    key = (seq_len, use_biases)
    if key not in _cache:
        _cache[key] = _build(seq_len, use_biases)
    nc = _cache[key]

    in_maps = _prep_inputs(inputs, seq_len, use_biases)
    res = run_bass_kernel_spmd(nc, in_maps, core_ids=list(range(NCORES)),
                               trace=TRACE)
    last_result = res

    b_O = np.asarray(inputs["b_O"], dtype=np.float32)
    out = np.zeros((B, seq_len, DM), np.float32)
    for c in range(NCORES):
        b = c // GROUPS
        out[b] += np.asarray(res.results[c]["out"], dtype=np.float32)
    out += b_O[None, None, :]
    return out


# revision 28
# speedup vs baseline: 1.0313x; 1.0066x over previous
"""Causal multi-head attention on 8 Trainium2 NeuronCores.

Problem: B=2, S=4096, D_MODEL=768, H=12, D_HEAD=64, fp32 I/O.

Sharding: (batch, head-group) -> core.  Cores 0-3 take batch 0, cores 4-7
take batch 1; each core computes 3 of the 12 heads for its batch and emits a
partial output [S, D_MODEL] (its heads' contribution to the W_O contraction)
in bf16.  The host sums the 4 partials per batch and adds b_O.

v3 design (vs v2): trace showed PE active 292us / ACT 233us of a 345us span,
with 26us startup, 73us of ACT gaps and a 23us half-clock tail.  Fixes:
  1. Startup: DMA order is idb|wqkv|xT-c0|wv3|...|xT-c1 so the first
     projection can start as soon as chunk 0 lands; the HAM warm-up spins on
     the zc PSUM bank (v2 used fill_ps, so proj #1 queued behind warm-up #40)
     and xT chunks >= 2 are DMA'd just-in-time at the end of window qs-2.
  2. ACT gaps: proj filler thunks split 6 matmuls -> 3x2 so a pop never
     exceeds the per-key-tile ACT slack; vproj split 2x3.
  3. Window-boundary gaps: norm_stage1 is split per head and h0/h1 copies
     are emitted before the h2 key loop, freeing the zab PSUM bank a full
     h2-loop earlier for the next window's AV accumulation.
  4. Junk trim: h1 (and h2's odd key tile) scores are written at shifted
     columns [QCH, 2QCH-vs) so diagonal-window exp calls shrink by vs on
     both ends (~8.5us ACT + ~8.5us PE).
  5. bf16 rank-1 1/r broadcast (f32r rhs streamed at half PE rate).
  6. Tail: anti-throttle dummy matmuls gated on the last window's norm
     chain keep the clock at 2.4GHz through the epilogue.
"""

import numpy as np
import ml_dtypes

B, S, DM, H, DH = 2, 4096, 768, 12, 64
NCORES = 8
GROUPS = 4                  # head-groups per batch
HPC = H // GROUPS           # heads per core = 3
P = 128
QCH = 512                   # psum bank width (fp32)

_BF = ml_dtypes.bfloat16

_cache = {}


def _build(seq_len, use_biases):
    import concourse.bacc as bacc
    import concourse.mybir as mybir
    import concourse.tile as tile

    f32 = mybir.dt.float32
    bf16 = mybir.dt.bfloat16
    Exp = mybir.ActivationFunctionType.Exp
    mult = mybir.AluOpType.mult
    add = mybir.AluOpType.add

    SQ = seq_len
    n_kt = SQ // P               # k tiles
    n_ch = SQ // QCH             # 512-wide chunks
    DSL = DM // P                # contraction slices for the projections
    KPC = QCH // P               # key tiles per chunk (4)

    nc = bacc.Bacc(None, target_bir_lowering=False)

    xT = nc.declare_dram_parameter("xT", [DM, SQ], bf16, isOutput=False)
    # packed projection weights: [Q01 | K01 | Q2K2] = 384 cols
    wqkv = nc.declare_dram_parameter("wqkv", [DM, 384], bf16, isOutput=False)
    # V weights for the direct [keys, z] projection, all 3 heads
    wv3 = nc.declare_dram_parameter("wv3", [DM, HPC * DH], bf16,
                                    isOutput=False)
    wo2 = nc.declare_dram_parameter("wo2", [P, DM], bf16, isOutput=False)
    wos = nc.declare_dram_parameter("wos", [DH, DM], bf16, isOutput=False)
    trimask = nc.declare_dram_parameter("trimask", [P, P], bf16, isOutput=False)
    ident_b = nc.declare_dram_parameter("ident_b", [P, P], bf16, isOutput=False)
    ones_z = nc.declare_dram_parameter("ones_z", [1, DH], bf16, isOutput=False)
    if use_biases:
        bqkv = nc.declare_dram_parameter("bqkv", [P, 3], f32, isOutput=False)
        bvrep = nc.declare_dram_parameter("bvrep", [P, HPC * DH], f32,
                                          isOutput=False)
    out = nc.declare_dram_parameter("out", [SQ, DM], bf16, isOutput=True)

    with tile.TileContext(nc) as tc:
        with (
            tc.tile_pool(name="singles", bufs=1) as singles,
            tc.tile_pool(name="persist", bufs=1) as persist,
            tc.tile_pool(name="nrm_t", bufs=2) as nrm_t,
            tc.tile_pool(name="nrm_k", bufs=6) as nrm_k,
            tc.tile_pool(name="xT_pool", bufs=1) as xT_pool,
            tc.tile_pool(name="s_ps", bufs=2, space="PSUM") as s_ps,
            tc.tile_pool(name="zab_ps", bufs=1, space="PSUM") as zab_ps,
            tc.tile_pool(name="zc_ps", bufs=1, space="PSUM") as zc_ps,
            tc.tile_pool(name="fill_ps", bufs=1, space="PSUM") as fill_ps,
            tc.tile_pool(name="pt_sb", bufs=4) as pt_pool,
            tc.tile_pool(name="o_sb", bufs=8) as o_pool,
        ):
            # ---- constants / weights, ordered for fastest first proj ----
            idb_sb = singles.tile([P, P], bf16)
            nc.sync.dma_start(idb_sb[:], ident_b[:])
            w_sb = singles.tile([P, DSL, 384], bf16, tag="wqkv")
            nc.sync.dma_start(w_sb[:], wqkv.rearrange("(o p) c -> p o c", p=P))
            xT_sb = xT_pool.tile([P, DSL, SQ], bf16)
            nc.sync.dma_start(
                xT_sb[:, :, 0:QCH],
                xT.rearrange("(o p) c -> p o c", p=P)[:, :, 0:QCH])
            wv_sb = singles.tile([P, DSL, HPC * DH], bf16, tag="wv3")
            nc.sync.dma_start(wv_sb[:], wv3.rearrange("(o p) c -> p o c", p=P))
            tri_sb = singles.tile([P, P], bf16)
            nc.sync.dma_start(tri_sb[:], trimask[:])
            ones_sb = singles.tile([1, DH], bf16)
            nc.sync.dma_start(ones_sb[:], ones_z[:])
            nc.sync.dma_start(
                xT_sb[:, :, QCH:2 * QCH],
                xT.rearrange("(o p) c -> p o c", p=P)[:, :, QCH:2 * QCH])
            wo2_sb = singles.tile([P, DM], bf16)
            nc.sync.dma_start(wo2_sb[:], wo2[:])
            wos_sb = singles.tile([DH, DM], bf16)
            nc.sync.dma_start(wos_sb[:], wos[:])
            for c0 in range(2, min(4, n_ch)):
                nc.sync.dma_start(
                    xT_sb[:, :, c0 * QCH:(c0 + 1) * QCH],
                    xT.rearrange("(o p) c -> p o c", p=P)[
                        :, :, c0 * QCH:(c0 + 1) * QCH])
            bias_sb = bv_sb = None
            if use_biases:
                bias_sb = singles.tile([P, 3], f32, tag="bias")
                nc.sync.dma_start(bias_sb[:], bqkv[:])
                bv_sb = singles.tile([P, HPC * DH], f32, tag="bvrep")
                nc.sync.dma_start(bv_sb[:], bvrep[:])

            # ---- persistent activations ----
            QT2 = persist.tile([P, SQ], bf16, tag="QT2")   # heads 0,1 stacked
            KT2 = persist.tile([P, SQ], bf16, tag="KT2")
            # head 2's Q/K live on BOTH partition halves so even/odd key
            # tiles can run concurrently in the two PE row groups
            Q2b = persist.tile([P, SQ], bf16, tag="Q2b")
            K2b = persist.tile([P, SQ], bf16, tag="K2b")
            V_sb = persist.tile([P, HPC, n_kt, DH + 1], bf16, tag="V")
            Zn2 = persist.tile([P, SQ], bf16, tag="Zn2")   # h0 rows 0-63, h1 64-127
            Zns = persist.tile([DH, SQ], bf16, tag="Zns")  # h2
            # single-instance scratch for the row-sum reciprocal: rows past
            # the written ones are read by the 32x32 block transposes, so
            # they are zeroed once and the tiles reused in place
            r32g = persist.tile([32, QCH], f32, tag="r32g")
            rrTg = persist.tile([32, QCH], f32, tag="rrTg")

            nc.vector.memset(V_sb[:, :, :, DH:DH + 1], 1.0)
            nc.vector.memset(r32g[:], 1.0)
            nc.vector.memset(rrTg[:], 1.0)

            # HAM warm-up on the zc bank (fill_ps stays free for the first
            # projections); zc's first real use is window 0's h2 AV, long
            # after this drains.  Kept short: the clock grant arrives ~5us
            # after PE activity starts, and dense MACs at full clock burn
            # the HAM utilization budget (16.4us half-clock quanta).
            wup = zc_ps.tile([DH + 1, QCH], f32, tag="zc", name="wup")
            for _ in range(40):
                nc.tensor.matmul(wup[:, 0:P], lhsT=idb_sb[:, 0:DH + 1],
                                 rhs=idb_sb[:], start=True, stop=True)

            # ================= projection thunks =================
            # groups: 0=Q01, 1=K01, 2=Q2|K2; V is projected per key-tile
            # directly in [keys, z] layout (lhsT = xT slice).
            def proj_mm(g, c, st, ps_slot):
                """Two of the six contraction matmuls for group g chunk c."""
                cs = slice(c * QCH, (c + 1) * QCH)
                if st == 0:
                    ps_slot[0] = fill_ps.tile([P, QCH], f32, tag="fill",
                                              name="proj_ps")
                ps = ps_slot[0]
                for o in (2 * st, 2 * st + 1):
                    nc.tensor.matmul(
                        ps[:], lhsT=w_sb[:, o, g * P:(g + 1) * P],
                        rhs=xT_sb[:, o, cs],
                        start=(o == 0), stop=(o == DSL - 1))
                if st < 2:
                    return
                if use_biases:
                    def cp(dst, src, brow):
                        nc.vector.tensor_scalar(
                            dst, src, bias_sb[brow, g:g + 1], None, add)
                else:
                    def cp(dst, src, brow=None):
                        nc.vector.tensor_copy(dst, src)
                if g == 0:
                    cp(QT2[:, cs], ps[:], slice(0, P))
                elif g == 1:
                    cp(KT2[:, cs], ps[:], slice(0, P))
                else:
                    cp(Q2b[0:DH, cs], ps[0:DH], slice(0, DH))
                    cp(K2b[DH:P, cs], ps[DH:P], slice(DH, P))
                    nc.sync.dma_start(Q2b[DH:P, cs], Q2b[0:DH, cs])
                    nc.sync.dma_start(K2b[0:DH, cs], K2b[DH:P, cs])

            def vproj_mm(kt, st, ps_slot):
                """Half of V-proj for one key-tile: [128 keys, 192]."""
                if st == 0:
                    ps_slot[0] = fill_ps.tile([P, QCH], f32, tag="fill",
                                              name="vproj_ps")[:, 0:HPC * DH]
                ps = ps_slot[0]
                for o in range(3 * st, 3 * st + 3):
                    nc.tensor.matmul(
                        ps[:], lhsT=xT_sb[:, o, kt * P:(kt + 1) * P],
                        rhs=wv_sb[:, o, :],
                        start=(o == 0), stop=(o == DSL - 1))
                if st == 0:
                    return
                dst = V_sb[:, :, kt, 0:DH]
                src = ps.rearrange("p (h z) -> p h z", z=DH)
                if use_biases:
                    nc.vector.tensor_tensor(
                        dst, src, bv_sb.rearrange("p (h z) -> p h z", z=DH),
                        add)
                else:
                    nc.vector.tensor_copy(dst, src)

            def proj_thunks(c):
                for g in range(3):
                    slot = [None]
                    for st in range(3):
                        yield (lambda g=g, c=c, st=st, slot=slot:
                               proj_mm(g, c, st, slot))
                for j in range(KPC):
                    slot = [None]
                    for st in range(2):
                        yield (lambda kt=c * KPC + j, st=st, slot=slot:
                               vproj_mm(kt, st, slot))

            # ---- normalization helpers (DVE stage + deferred PE stage) ----
            def stage1(zacc):
                """All-DVE per head: copy Z to SBUF (freeing the PSUM bank),
                extract row sums, reciprocal via 32x32 block transposes."""
                zsb = nrm_k.tile([DH, QCH], bf16, tag="zsb", name="zsb")
                nc.vector.tensor_copy(zsb[:], zacc[0:DH, :])
                nc.vector.tensor_copy(r32g[0:1, :], zacc[DH:DH + 1, :])
                rT = nrm_t.tile([32, QCH], f32, tag="rT", name="rT")
                nc.vector.transpose(rT[:], r32g[:])
                nc.vector.reciprocal(
                    rrTg.rearrange("p (j c) -> p j c", c=32)[:, :, 0],
                    rT.rearrange("p (j c) -> p j c", c=32)[:, :, 0])
                rr32 = nrm_t.tile([32, QCH], f32, tag="rr32", name="rr32")
                nc.vector.transpose(rr32[:], rrTg[:])
                rr = nrm_k.tile([1, QCH], bf16, tag="rr", name="rr_sb")
                nc.vector.tensor_copy(rr[:], rr32[0:1, :])
                return (rr, zsb)

            def norm_stage2(h, q0, staged):
                """PE rank-1 broadcast of 1/r, then one DVE multiply."""
                rr_sb, zsb = staged
                rrb = fill_ps.tile([P, QCH], f32, tag="fill",
                                   name="rrb")[0:DH]
                nc.tensor.matmul(rrb[:], lhsT=ones_sb[:],
                                 rhs=rr_sb[:], start=True, stop=True)
                if h == 0:
                    nc.vector.tensor_tensor(
                        Zn2[0:DH, q0:q0 + QCH], zsb[:], rrb[:], mult)
                elif h == 1:
                    t = nrm_k.tile([DH, QCH], bf16, tag="zn1", name="zn1")
                    nc.vector.tensor_tensor(t[:], zsb[:], rrb[:], mult)
                    nc.sync.dma_start(Zn2[DH:P, q0:q0 + QCH], t[:])
                else:
                    nc.vector.tensor_tensor(
                        Zns[:, q0:q0 + QCH], zsb[:], rrb[:], mult)

            # ===== flash: all heads interleaved, one 512-wide window loop ====
            SW = 2 * QCH            # psum score-slot width (tag "S")
            HD = DM // 2
            fills = []

            def oproj_thunks(w, tail=False):
                """O-proj for window w as per-half-tile filler thunks:
                h01 packed (contraction 128) + h2 (contraction 64).  In the
                post-loop tail the score banks are dead, so po rotates
                through s_ps (bufs=2) instead of serializing every
                matmul/cast pair through the single fill bank."""
                thunks = []
                for tt in range(w * (QCH // P), (w + 1) * (QCH // P)):
                    osb = o_pool.tile([P, DM], bf16, tag="osb", name="osb")

                    def th(tt=tt, osb=osb, half=0, tl=tail):
                        if tl:
                            po = s_ps.tile([P, 2 * QCH], f32, tag="S",
                                           name="po_t")[:, 0:HD]
                        else:
                            po = fill_ps.tile([P, QCH], f32, tag="fill",
                                              name="po")[:, 0:HD]
                        hs = slice(half * HD, (half + 1) * HD)
                        nc.tensor.matmul(
                            po[:], lhsT=Zn2[:, tt * P:(tt + 1) * P],
                            rhs=wo2_sb[:, hs], start=True, stop=False)
                        nc.tensor.matmul(
                            po[:], lhsT=Zns[:, tt * P:(tt + 1) * P],
                            rhs=wos_sb[:, hs], start=False, stop=True)
                        if tl:
                            # tail: ACT is idle after the last exp, and the
                            # serial DVE cast chain is the tail's critical
                            # path (Copy is in every ACT table set)
                            nc.scalar.copy(osb[:, hs], po[:])
                        else:
                            nc.vector.tensor_copy(osb[:, hs], po[:])
                        if half == 1:
                            nc.sync.dma_start(out[tt * P:(tt + 1) * P, :],
                                              osb[:])

                    thunks.append(th)
                    thunks.append(
                        lambda tt=tt, osb=osb, th=th: th(tt, osb, 1))
                return thunks

            # Ascending windows; proj chunks {0,1} up front, chunk qs+2
            # rides window qs's fills.  stage2/oproj for window qs-1 also
            # ride window qs, with oproj queued after proj so its stage2 /
            # Zn-shift-DMA inputs are settled by the time it pops.
            # chunks 0-3 are projected up front: ACT is idle during the
            # (half-clock) startup anyway, and this keeps the early windows'
            # filler load from starving the score matmuls that feed exp
            for c in range(min(4, n_ch)):
                for th in proj_thunks(c):
                    th()

            staged = {}
            prev = None
            for qs in range(n_ch):
                q0 = qs * QCH
                # just-in-time xT stream: chunk qs+4's DMA is emitted here,
                # before its projection fills are queued below
                if qs + 4 < n_ch:
                    c = qs + 4
                    nc.sync.dma_start(
                        xT_sb[:, :, c * QCH:(c + 1) * QCH],
                        xT.rearrange("(o p) c -> p o c", p=P)[
                            :, :, c * QCH:(c + 1) * QCH])
                if prev is not None:
                    for h in range(HPC):
                        fills.append(
                            (lambda h=h, q=prev * QCH, st=staged[(prev, h)]:
                             norm_stage2(h, q, st)))
                if qs + 4 < n_ch:
                    fills.extend(proj_thunks(qs + 4))
                if prev is not None:
                    fills.extend(oproj_thunks(prev))

                zab = zab_ps.tile([DH + 1, 2 * QCH], f32, tag="zab",
                                  name="zab")
                za = zab[:, 0:QCH]
                zb = zab[:, QCH:2 * QCH]
                zc = zc_ps.tile([DH + 1, QCH], f32, tag="zc", name="zc")
                nk = KPC * qs + KPC
                # --- heads 0,1: concurrent scores in two PE row groups;
                # h1 columns are shifted by -vs on diagonal tiles so the
                # joint exp call is junk-free on both ends ---
                for ki in range(nk):
                    vs = max(0, P * ki - q0)
                    ssc = s_ps.tile([P, SW], f32, tag="S", name="ssc")
                    nc.tensor.matmul(
                        ssc[:, vs:QCH],
                        lhsT=KT2[0:DH, ki * P:(ki + 1) * P],
                        rhs=QT2[0:DH, q0 + vs:q0 + QCH],
                        start=True, stop=True)
                    nc.tensor.matmul(
                        ssc[:, QCH:2 * QCH - vs],
                        lhsT=KT2[DH:P, ki * P:(ki + 1) * P],
                        rhs=QT2[DH:P, q0 + vs:q0 + QCH],
                        start=True, stop=True)
                    pt = pt_pool.tile([P, 2 * QCH], bf16, tag="PT",
                                      name="pt")
                    nc.scalar.activation(
                        pt[:, vs:2 * QCH - vs], ssc[:, vs:2 * QCH - vs],
                        Exp, scale=0.125)
                    if ki >= KPC * qs:  # diagonal tile: mask both heads
                        nc.vector.tensor_tensor(
                            pt[:, vs:vs + P], pt[:, vs:vs + P],
                            tri_sb[:], mult)
                        nc.vector.tensor_tensor(
                            pt[:, QCH:QCH + P], pt[:, QCH:QCH + P],
                            tri_sb[:], mult)
                    nc.tensor.matmul(
                        za[:, vs:QCH], lhsT=V_sb[:, 0, ki, :],
                        rhs=pt[:, vs:QCH],
                        start=(ki == 0), stop=(ki == nk - 1))
                    nc.tensor.matmul(
                        zb[:, vs:QCH], lhsT=V_sb[:, 1, ki, :],
                        rhs=pt[:, QCH:2 * QCH - vs],
                        start=(ki == 0), stop=(ki == nk - 1))
                    if fills:
                        fills.pop(0)()
                # za/zb are complete: free the zab bank for the next window
                # before the h2 loop runs (DVE copies overlap h2's ACT time)
                staged[(qs, 0)] = stage1(za)
                staged[(qs, 1)] = stage1(zb)
                if qs == n_ch - 1:
                    # last window: h0/h1 stage2 ride the h2 loop's fill pops
                    # so the post-loop tail only carries the h2 chain
                    for h in range(2):
                        fills.append(
                            (lambda h=h, q=q0, st=staged[(qs, h)]:
                             norm_stage2(h, q, st)))
                # --- head 2: two key-tiles per exp call ---
                for kj in range(0, nk, 2):
                    vs0 = max(0, P * kj - q0)
                    vs1 = max(0, P * (kj + 1) - q0)
                    ssc = s_ps.tile([P, SW], f32, tag="S", name="ssc2")
                    nc.tensor.matmul(
                        ssc[:, vs0:QCH],
                        lhsT=K2b[0:DH, kj * P:(kj + 1) * P],
                        rhs=Q2b[0:DH, q0 + vs0:q0 + QCH],
                        start=True, stop=True)
                    nc.tensor.matmul(
                        ssc[:, QCH:2 * QCH - vs1],
                        lhsT=K2b[DH:P, (kj + 1) * P:(kj + 2) * P],
                        rhs=Q2b[DH:P, q0 + vs1:q0 + QCH],
                        start=True, stop=True)
                    pt = pt_pool.tile([P, 2 * QCH], bf16, tag="PT2",
                                      name="pt2")
                    nc.scalar.activation(
                        pt[:, vs0:2 * QCH - vs1], ssc[:, vs0:2 * QCH - vs1],
                        Exp, scale=0.125)
                    if kj >= KPC * qs:  # both tiles on the diagonal
                        nc.vector.tensor_tensor(
                            pt[:, vs0:vs0 + P], pt[:, vs0:vs0 + P],
                            tri_sb[:], mult)
                        nc.vector.tensor_tensor(
                            pt[:, QCH:QCH + P], pt[:, QCH:QCH + P],
                            tri_sb[:], mult)
                    nc.tensor.matmul(
                        zc[:, vs0:QCH], lhsT=V_sb[:, 2, kj, :],
                        rhs=pt[:, vs0:QCH],
                        start=(kj == 0), stop=False)
                    nc.tensor.matmul(
                        zc[:, vs1:QCH], lhsT=V_sb[:, 2, kj + 1, :],
                        rhs=pt[:, QCH:2 * QCH - vs1],
                        start=False, stop=(kj + 1 == nk - 1))
                    if fills:
                        fills.pop(0)()
                staged[(qs, 2)] = stage1(zc)
                prev = qs
            while fills:
                fills.pop(0)()
            norm_stage2(2, prev * QCH, staged[(prev, 2)])
            for th in oproj_thunks(prev, tail=True):
                th()

    nc.compile()
    return nc


def _prep_inputs(inputs, seq_len, use_biases):
    x = np.asarray(inputs["normalized_resid_pre"], dtype=np.float32)
    WQ = np.asarray(inputs["W_Q"], dtype=np.float32)
    WK = np.asarray(inputs["W_K"], dtype=np.float32)
    WV = np.asarray(inputs["W_V"], dtype=np.float32)
    WO = np.asarray(inputs["W_O"], dtype=np.float32)

    tri = np.triu(np.ones((P, P), np.float32)).astype(_BF)  # keep j >= p
    idb = np.eye(P, dtype=np.float32).astype(_BF)
    onz = np.ones((1, DH), np.float32).astype(_BF)

    in_maps = []
    for c in range(NCORES):
        b, g = divmod(c, GROUPS)
        hs = slice(g * HPC, (g + 1) * HPC)
        wq = WQ[hs]   # [3, DM, DH]
        wk = WK[hs]
        wv = WV[hs]
        wo = WO[hs]   # [3, DH, DM]
        # packed groups: [Q01 | K01 | Q2K2] -> [DM, 384]
        wqkv = np.concatenate([
            wq[0], wq[1], wk[0], wk[1], wq[2], wk[2],
        ], axis=1)
        wv3 = np.concatenate([wv[0], wv[1], wv[2]], axis=1)
        m = {
            "xT": np.ascontiguousarray(x[b, :seq_len].T).astype(_BF),
            "wqkv": np.ascontiguousarray(wqkv).astype(_BF),
            "wv3": np.ascontiguousarray(wv3).astype(_BF),
            "wo2": np.ascontiguousarray(
                np.concatenate([wo[0], wo[1]], axis=0)).astype(_BF),
            "wos": np.ascontiguousarray(wo[2]).astype(_BF),
            "trimask": tri,
            "ident_b": idb,
            "ones_z": onz,
        }
        if use_biases:
            bq = np.asarray(inputs["b_Q"], np.float32)[hs]
            bk = np.asarray(inputs["b_K"], np.float32)[hs]
            bv = np.asarray(inputs["b_V"], np.float32)[hs]
            bias = np.zeros((P, 3), np.float32)
            bias[:, 0] = np.concatenate([bq[0], bq[1]])
            bias[:, 1] = np.concatenate([bk[0], bk[1]])
            bias[:, 2] = np.concatenate([bq[2], bk[2]])
            m["bqkv"] = bias
            m["bvrep"] = np.broadcast_to(
                bv.reshape(1, HPC * DH), (P, HPC * DH)).copy()
        in_maps.append(m)
    return in_maps


TRACE = False          # test.py can flip this to get exec_time_ns
last_result = None     # BassKernelResults of the most recent run


def kernel(seq_len=S, **inputs):
    global last_result
    from concourse.bass_utils import run_bass_kernel_spmd

    use_biases = any(
        np.any(np.asarray(inputs[k]) != 0) for k in ("b_Q", "b_K", "b_V"))

    key = (seq_len, use_biases)
    if key not in _cache:
        _cache[key] = _build(seq_len, use_biases)
    nc = _cache[key]

    in_maps = _prep_inputs(inputs, seq_len, use_biases)
    res = run_bass_kernel_spmd(nc, in_maps, core_ids=list(range(NCORES)),
                               trace=TRACE)
    last_result = res

    b_O = np.asarray(inputs["b_O"], dtype=np.float32)
    out = np.zeros((B, seq_len, DM), np.float32)
    for c in range(NCORES):
        b = c // GROUPS
        out[b] += np.asarray(res.results[c]["out"], dtype=np.float32)
    out += b_O[None, None, :]
    return out
